# revision 26
# baseline (speedup 1.0000x reference)
"""Trainium2 Bass kernel for nn_LocalRNN: 8-step CTRNN over sliding windows.

Math:
  For each position l: h_{k+1} = a*h_k + relu(h_k @ W* + u*[l+k]),  h_0 = 0
  where a = 1 - 1/tau, W* = W * (1/tau) (columns), u* = Xp @ W_in* + b*,
  W_in* = W_in * (1/tau), b* = b * (1/tau).  Output = h_8 per position.
  (Uses relu(c*z) = c*relu(z) for c>0 to fold 1/tau into the weights, and
  the fact that the input projection is shared across overlapping windows.)

Sharding: batch dim (8) across the 8 NeuronCores, weights replicated.
On-chip layout is transposed ([d on partitions, positions on free dim]) so
matmuls contract d on the partition axis; the host uploads x pre-transposed
in bf16 and transposes the bf16 [d, pos] output back (layout marshalling).

Everything is bf16 (PE runs bf16 at 1 cyc/col like f32r, but DVE gets its
2-byte fast modes and DMA bytes halve). Per step, each of the four
[128,1024] PSUM tiles gets: 2 identity matmuls injecting u (only PE writes
accumulate in PSUM), 4 W-block matmuls, an ACT relu to bf16, and an
h-update h' = a*h + r done as DVE tensor_scalar (prefetched) +
tensor_tensor.

Head: DMA rings have ~1.5-3.7us issue-to-first-packet latency, so the
consts blob goes first on the fastest-waking ring (sync) and x is split
into column pieces on two more rings (scalar: d-block 0, vector: d-block
1) so the u-projection — reorganised into 512-col quarter chunks, two
quarters sharing one PSUM tile — starts on the first x piece instead of
waiting for all of x.  Graduated dummy matmuls bridge the PE clock ramp
across the wait.

Tail: step 7 runs all four tiles at 512 granularity and the output leaves
eagerly in chunk-major DRAM layout (NW, D, WCH) so every piece is a
contiguous DRAM region: three [128,1024] pieces stream on the sync HW
ring largely hidden under the remaining compute, and the final tile goes
out as two 512-col halves back-to-back on the scalar HW ring so the first
half transfers while the second half's tensor_tensor completes.
Ring-warming dummy DMAs (data-gated on late h so the scheduler cannot
hoist them) bridge each ring's ~1.3us wake latency.
"""

import numpy as np
import ml_dtypes
from contextlib import ExitStack

import concourse.bass as bass
import concourse.tile as tile
from concourse import bacc, mybir
from concourse.bass_utils import run_bass_kernel_spmd

B, L, D, KSIZE = 8, 2048, 256, 8
P = 128
NCORES = 8
MMN = 512                    # matmul moving free dim (PSUM bank limit)
WCH = 1024                   # tile width for PSUM tiles / elementwise ops
NW = L // WCH                # 2
UCOLS = L + KSIZE - 1        # 2055
PAD = KSIZE - 1              # 7
DB = D // P                  # 2 d-blocks
NQ = L // MMN                # 4 u-proj quarter chunks
F32 = mybir.dt.float32
BF16 = mybir.dt.bfloat16
AF = mybir.ActivationFunctionType
ALU = mybir.AluOpType
BF16NP = ml_dtypes.bfloat16

# packed bf16 const blob: wint0|wint1|wt0|wt1|identity (one DMA, wide lines)
CW_COLS = 4 * D + P
CW_W = 2 * D
CW_ID = 4 * D
# packed f32 consts blob: bst | at | pad src
CF_COLS = 2 * DB + PAD + 1
_cache = {}

# --- tuning ---
# Generous bridge: undershoot leaves a PE-idle gap that resets the HAM
# activity window (clock stuck at 1.2 GHz for ~3.4us more — catastrophic),
# overshoot only wastes ~90ns per extra small matmul.
N_WARM_BIG = 10           # 512-col dummy matmuls (~427ns spacing cold)
N_WARM_SMALL = 8          # 128-col dummy matmuls (~107ns spacing cold)


def _build_program():
    nc = bacc.Bacc(
        "TRN2",
        target_bir_lowering=False,
        debug=False,
        enable_asserts=False,
        num_devices=NCORES,
    )
    # x uploaded pre-transposed bf16: (D, L), row d -> [d, positions]
    x_d = nc.dram_tensor("xt", (D, L), BF16, kind="ExternalInput").ap()
    cw_d = nc.dram_tensor("constsw", (P, CW_COLS), BF16, kind="ExternalInput").ap()
    cf_d = nc.dram_tensor("constsf", (P, CF_COLS), F32, kind="ExternalInput").ap()
    # output in chunk-major T-layout bf16: (NW, D, WCH) so each output
    # piece is one fully contiguous DRAM region; host reassembles
    out_d = nc.dram_tensor("out", (NW, D, WCH), BF16,
                           kind="ExternalOutput").ap()
    # scratch target for ring-warming writes (absorbs the wake latency of a
    # DMA ring before the real output transfers)
    scr_d = nc.dram_tensor("scr", (P, 24), BF16, kind="Internal").ap()

    with tile.TileContext(nc) as tc, ExitStack() as ctx:
        consts = ctx.enter_context(tc.tile_pool(name="consts", bufs=1))
        big = ctx.enter_context(tc.tile_pool(name="big", bufs=1))
        rp = ctx.enter_context(tc.tile_pool(name="rp", bufs=4))
        ahp = ctx.enter_context(tc.tile_pool(name="ahp", bufs=4))
        # single PSUM pool: [128,1024] slot = 2 banks, bufs=4 -> all 8 banks
        zp = ctx.enter_context(tc.tile_pool(name="zp", bufs=4, space="PSUM"))

        # --- constants ---
        cw = consts.tile([P, CW_COLS], BF16, name="cw")
        cf = consts.tile([P, CF_COLS], F32, name="cf")
        wint = [cw[:, i * D:(i + 1) * D] for i in range(DB)]
        wt = [cw[:, CW_W + i * D:CW_W + (i + 1) * D] for i in range(DB)]
        identb = cw[:, CW_ID:CW_ID + P]
        bst = cf[:, 0:DB]
        at = cf[:, DB:2 * DB]
        padsrc = cf[:, 2 * DB:2 * DB + PAD]

        # --- persistent buffers (bf16) ---
        # x as one [p, i*L + c] tile; pieces arrive per column quarter
        xall = big.tile([P, DB * L], BF16, name="xall")
        ut = [big.tile([P, UCOLS], BF16, name=f"ut{i}") for i in range(DB)]
        hball = [big.tile([P, DB * L], BF16, name=f"hb{s}") for s in range(2)]
        hb = [[hball[s][:, i * L:(i + 1) * L] for i in range(DB)]
              for s in range(2)]
        h1 = hb[1]

        # --- input DMAs. The head is input-fabric-bound (~1.3MB at
        # ~310-330 GB/s aggregate after the first ring wakes ~8.7us), so
        # what matters is (a) balancing bytes across the two HW rings and
        # (b) delivering in need order: cf + W_in first (they gate the
        # first u-proj matmul), the x first halves next, W/identity before
        # step 1, the x second halves last (u-proj gw0 overlaps their
        # arrival).
        xsrc = x_d.rearrange("(i p) c -> p i c", p=P)
        nc.sync.dma_start(cf[:], cf_d[:, :])
        nc.sync.dma_start(cw[:, 0:CW_W], cw_d[:, 0:CW_W])
        nc.scalar.dma_start(xall[:, 0:WCH], xsrc[:, 0, 0:WCH])
        nc.sync.dma_start(xall[:, L:L + WCH], xsrc[:, 1, 0:WCH])
        nc.scalar.dma_start(cw[:, CW_W:CW_COLS], cw_d[:, CW_W:CW_COLS])
        nc.scalar.dma_start(xall[:, WCH:L], xsrc[:, 0, WCH:L])
        nc.sync.dma_start(xall[:, L + WCH:2 * L], xsrc[:, 1, WCH:L])

        # --- PE warmup: dummy matmuls on zeros to engage the clock early.
        dummy = big.tile([P, MMN], BF16, name="dummy")
        nc.vector.memset(dummy[:], 0.0)
        for w in range(N_WARM_BIG + N_WARM_SMALL):
            warm = zp.tile([P, WCH], F32, name="warm", tag="z")
            wn = MMN if w < N_WARM_BIG else P
            nc.tensor.matmul(warm[:, 0:wn], lhsT=dummy[:, 0:P],
                             rhs=dummy[:, 0:wn], start=True, stop=True)

        # u pad cols + h1 pad cols (also warms the ACT table early):
        # u[:, :7] = b*, h1[:, :7] = relu(b*)
        for j in range(DB):
            nc.scalar.activation(
                ut[j][:, 0:PAD], padsrc,
                AF.Identity, bias=bst[:, j:j + 1], scale=0.0,
            )
            nc.scalar.activation(
                h1[j][:, 0:PAD], padsrc,
                AF.Relu, bias=bst[:, j:j + 1], scale=0.0,
            )

        # --- u projection: 4 tiles (gw, j) gated by the x half-pieces;
        # i-outer order shares LDWEIGHTS across the two 512 halves. Post
        # ops split between ACT and DVE: ACT takes every h1 relu (step-1's
        # W-matmuls need h1 first) and DVE every u-store, so neither chain
        # queues behind the other at the u -> step-1 handoff.
        for gw in range(2):
            for j in range(DB):
                zt = zp.tile([P, WCH], F32, name="zu", tag="z")
                for i in range(DB):
                    for half in range(2):
                        xs = i * L + gw * WCH + half * MMN
                        nc.tensor.matmul(
                            zt[:, half * MMN:(half + 1) * MMN],
                            lhsT=wint[i][:, j * P:(j + 1) * P],
                            rhs=xall[:, xs:xs + MMN],
                            start=(i == 0),
                            stop=(i == DB - 1),
                        )
                us = PAD + gw * WCH
                hw = WCH if gw == 0 else WCH - PAD
                nc.scalar.activation(
                    h1[j][:, us:us + hw], zt[:, 0:hw],
                    AF.Relu, bias=bst[:, j:j + 1], scale=1.0,
                )
                nc.vector.tensor_scalar(
                    out=ut[j][:, us:us + WCH], in0=zt[:],
                    scalar1=bst[:, j:j + 1], scalar2=None,
                    op0=ALU.add,
                )

        # --- steps 1..7 ---
        h8all = hball[KSIZE % 2]
        osrc = out_d.rearrange("n (i p) w -> p n i w", p=P)

        for k in range(1, KSIZE):
            hc = hb[k % 2]
            hn = hb[(k + 1) % 2]
            last_step = (k == KSIZE - 1)
            for c in range(NW):
                cs = c * WCH
                for j in range(DB):
                    zt = zp.tile([P, WCH], F32, name="zt", tag="z")
                    # identity matmul first (u is ready early), W blocks after
                    for half in range(2):
                        nc.tensor.matmul(
                            zt[:, half * MMN:(half + 1) * MMN],
                            lhsT=identb,
                            rhs=ut[j][:, k + cs + half * MMN:
                                      k + cs + half * MMN + MMN],
                            start=True, stop=False,
                        )
                    for i in range(DB):
                        for half in range(2):
                            hs = cs + half * MMN
                            nc.tensor.matmul(
                                zt[:, half * MMN:(half + 1) * MMN],
                                lhsT=wt[i][:, j * P:(j + 1) * P],
                                rhs=hc[i][:, hs:hs + MMN],
                                start=False,
                                stop=(i == DB - 1),
                            )
                    # relu + h-update (h' = a*h + r): the a-scale TS only
                    # needs the previous h so it runs early; only the TT add
                    # sits after the relu on the critical path. The whole
                    # last step runs at 512 granularity so each output
                    # piece's drain is short.
                    nsub = 2 if last_step else 1
                    sw = WCH // nsub
                    ah = ahp.tile([P, WCH], BF16, name="ah", tag="ah")
                    nc.vector.tensor_scalar(
                        out=ah[:], in0=hc[j][:, cs:cs + WCH],
                        scalar1=at[:, j:j + 1], scalar2=None,
                        op0=ALU.mult,
                    )
                    for s in range(nsub):
                        ss = s * sw
                        r = rp.tile([P, sw], BF16, name="r", tag="r")
                        if last_step and c == NW - 1 and j == DB - 1 and s == 0:
                            # very last tile: run this relu half on DVE so
                            # both halves drain the PSUM in parallel
                            nc.vector.tensor_scalar(
                                out=r[:], in0=zt[:, ss:ss + sw],
                                scalar1=0.0, scalar2=None, op0=ALU.max,
                            )
                        else:
                            nc.scalar.activation(
                                r[:], zt[:, ss:ss + sw], AF.Relu)
                        nc.vector.tensor_tensor(
                            hn[j][:, cs + ss:cs + ss + sw],
                            ah[:, ss:ss + sw], r[:], ALU.add,
                        )
                    # eager output: ship each [128,1024] piece right after
                    # its last TT. The first three pieces stream on the
                    # sync HW ring (mostly hidden under remaining compute);
                    # the last piece rides the scalar HW ring in parallel.
                    # Keep-alive scr writes (sourced from freshly written h
                    # so the scheduler cannot hoist them) bridge each
                    # ring's wake latency: rings fall asleep again within
                    # a few us of going idle.
                    if last_step:
                        if (c, j) == (1, 1):
                            # final tile: two 512-col halves back-to-back on
                            # the scalar ring — the first half transfers
                            # while the second half's TT completes
                            for s in range(nsub):
                                ss = cs + s * sw
                                nc.scalar.dma_start(
                                    osrc[:, c, j, s * sw:(s + 1) * sw],
                                    h8all[:, j * L + ss:j * L + ss + sw],
                                )
                        else:
                            nc.sync.dma_start(
                                osrc[:, c, j, :],
                                h8all[:, j * L + cs:j * L + cs + WCH],
                            )
                        if (c, j) == (0, 0):
                            # bridge scalar's wake so its c1j1 piece rides
                            # a warm ring
                            nc.scalar.dma_start(scr_d[:, 8:12],
                                                hn[0][:, 1020:1024])
                        if (c, j) == (1, 0):
                            nc.scalar.dma_start(scr_d[:, 12:16],
                                                hn[0][:, 2044:2048])
            # wake the sync output ring just before step 7's first piece
            if k == KSIZE - 2:
                nc.sync.dma_start(scr_d[:, 4:8], hn[1][:, L - 8:L - 4])

    nc.compile()
    return nc


def get_program():
    if "nc" not in _cache:
        _cache["nc"] = _build_program()
    return _cache["nc"]


def make_in_maps(x, weight, input_weight, bias, tau):
    x = np.asarray(x, dtype=np.float32)
    weight = np.asarray(weight, dtype=np.float32)
    input_weight = np.asarray(input_weight, dtype=np.float32)
    bias = np.asarray(bias, dtype=np.float32).reshape(1, D)
    tau = np.asarray(tau, dtype=np.float32).reshape(1, D)

    inv_tau = 1.0 / tau                       # (1, D)
    a = 1.0 - inv_tau
    wstar = (weight * inv_tau).astype(np.float32)          # scale columns
    winstar = (input_weight * inv_tau).astype(np.float32)
    bstar = (bias * inv_tau).astype(np.float32)
    # per-partition layout (P, DB): col j holds elems [j*P, (j+1)*P)
    bstar_t = bstar.reshape(DB, P).T
    a_t = a.reshape(DB, P).T
    ident = np.eye(P, dtype=np.float32)

    cwb = np.concatenate(
        [winstar[0:P, :], winstar[P:D, :], wstar[0:P, :], wstar[P:D, :],
         ident], axis=1)
    cf = np.concatenate(
        [bstar_t, a_t, np.zeros((P, PAD + 1), np.float32)], axis=1)

    shared = {
        "constsw": np.ascontiguousarray(cwb.astype(BF16NP)),
        "constsf": np.ascontiguousarray(cf),
    }
    return [
        {"xt": np.ascontiguousarray(x[b].T.astype(BF16NP)), **shared}
        for b in range(NCORES)
    ]


def kernel(x, weight, input_weight, bias, tau, ksize, _trace=False):
    assert int(ksize) == KSIZE
    nc = get_program()
    in_maps = make_in_maps(x, weight, input_weight, bias, tau)
    res = run_bass_kernel_spmd(
        nc, in_maps, core_ids=list(range(NCORES)), trace=_trace
    )
    out = np.stack(
        [np.vstack([res.results[b]["out"][0].T, res.results[b]["out"][1].T])
         for b in range(NCORES)],
        axis=0,
    )
    if _trace:
        _cache["last_results"] = res
    return out.astype(np.float32)
